# revision 10
# baseline (speedup 1.0000x reference)
"""Trainium2 Bass kernel for the NT-Xent / CLIP-style contrastive loss.

Reference computation (N=8192, D=512, fp32):
    zi_n, zj_n = row-normalize(z_i), row-normalize(z_j)
    sim = zi_n @ zj_n.T / TAU
    loss_e2t = mean_i( logsumexp_{j!=i}(sim[i,:]) - sim[i,i] )
    loss_t2e = mean_j( logsumexp_{i!=j}(sim[:,j]) - sim[j,j] )
    out = [ (loss_e2t+loss_t2e)/2, loss_e2t, loss_t2e ]

Sharding: rows of z_i are split across the 8 cores (1024 rows each); the
normalized z_j is replicated (the host plays the role of the all-gather).
Each core computes its [1024, 8192] tile of exp(sim), reducing it two ways:
  * row sums — fused into the ScalarE `activation(Exp, accum_out=...)`
  * col sums — partial per 128-partition group, accumulated on VectorE
    into a [128, 8192] buffer (the remaining 128-way + 8-core reduction
    is the host-side all-reduce)
The diagonal is NOT masked on device: since z_i != z_j the diagonal sims
are not outliers, so the host subtracts exp(pos) from the gathered sums
and finishes with log / means in float64.

Main matmul runs in fp8e4m3 with DoubleRow packing (2 contraction rows per
PE cell -> 0.5 cycles/row). Operands are scaled by 32 before the fp8 cast
to stay clear of denormals; the 1/32^2 is folded into the exp scale.
Set DT_MAIN="bf16" for the (slower, more precise) bf16 fallback.
"""

import os
import sys

for _p in ("/opt/trn_rl_repo", "/root/.axon_site/_ro/trn_rl_repo"):
    if os.path.isdir(_p) and _p not in sys.path:
        sys.path.insert(0, _p)

import numpy as np
import ml_dtypes

import concourse.bass as bass
import concourse.bacc as bacc
import concourse.mybir as mybir
import concourse.tile as tile
from concourse import bass_utils

TAU = 0.07
EPS = 1e-8

N = 8192            # batch
D = 512             # embed dim
NCORES = 8
NI = N // NCORES    # rows per core (1024)
P = 128             # partitions
RC = NI // P        # row chunks per core (8)
CCG = 2048          # columns per exp/accumulate group
NCCG = N // CCG     # 4 groups
MMN = 512           # matmul moving size (one PSUM bank of fp32)

DT_MAIN = os.environ.get("KERNEL_DT", "fp8")  # "fp8" | "bf16"
FP8_SCALE = 32.0

BF16 = mybir.dt.bfloat16
F32 = mybir.dt.float32
FP8 = mybir.dt.float8e4
NP_FP8 = mybir.dt.np(FP8)

LAST_RESULTS = None  # BassKernelResults of the most recent run (for test.py)

_compiled = {}


def _build():
    """Build + compile the single-core SPMD Bass program."""
    nc = bacc.Bacc("TRN2", target_bir_lowering=False, debug=False)

    if DT_MAIN == "fp8":
        # zi: [kk, p, slab, n] with contraction row d = kk*256 + slab*128 + p.
        # zj adds a group dim so each [g] chunk is contiguous per partition
        # (16KB runs -> full DMA bandwidth): [kk, g, p, slab, cols-in-group]
        zi_t = nc.dram_tensor("zi_t", [2, P, 2, NI], FP8, kind="ExternalInput")
        zj_t = nc.dram_tensor("zj_t", [2, NCCG, P, 2, CCG], FP8, kind="ExternalInput")
    else:
        zi_t = nc.dram_tensor("zi_t", [D, NI], BF16, kind="ExternalInput")
        zj_t = nc.dram_tensor("zj_t", [D, N], BF16, kind="ExternalInput")
    rows_d = nc.dram_tensor("rowsums", [P, RC * NCCG], F32, kind="ExternalOutput")
    cols_d = nc.dram_tensor("colacc", [P, N], BF16, kind="ExternalOutput")

    with tile.TileContext(nc) as tc:
        _body(nc, tc, zi_t.ap(), zj_t.ap(), rows_d.ap(), cols_d.ap())

    nc.compile()
    return nc


def _body(nc, tc, zi_t, zj_t, rows_d, cols_d):
    from contextlib import ExitStack

    fp8 = DT_MAIN == "fp8"
    kc = 2 if fp8 else 4  # contraction instruction count per output element
    exp_scale = 1.0 / (TAU * FP8_SCALE * FP8_SCALE) if fp8 else 1.0 / TAU
    perf_mode = mybir.MatmulPerfMode.DoubleRow if fp8 else None

    with ExitStack() as ctx:
        zpool = ctx.enter_context(tc.tile_pool(name="z", bufs=1))
        epool = ctx.enter_context(tc.tile_pool(name="e", bufs=3))
        apool = ctx.enter_context(tc.tile_pool(name="acc", bufs=1))
        psump = ctx.enter_context(
            tc.tile_pool(name="psum", bufs=2, space=bass.MemorySpace.PSUM)
        )

        # ---- PE clock warmup ------------------------------------------
        # ~10 dummy DoubleRow matmuls on a memset tile keep the PE busy
        # during the input DMA window so the HAM clock gate opens (1.2 ->
        # 2.4 GHz) before the first real matmul issues.
        if fp8:
            wsrc = zpool.tile([P, 2, MMN], FP8, tag="wsrc", name="wsrc")
            nc.gpsimd.memset(wsrc[:], 0)
            for w in range(10):
                wp = psump.tile([P, MMN], F32, tag="G", name="warm")
                nc.tensor.matmul(
                    wp[:],
                    wsrc[:, :, 0:P],
                    wsrc[:],
                    start=True,
                    stop=True,
                    perf_mode=perf_mode,
                )

        # ---- stage inputs in SBUF -------------------------------------
        # Alternate the two DMA paths (HWDGE via sync, SWDGE via gpsimd)
        # so transfers overlap instead of serializing on one queue.
        if fp8:
            zi_sb = [
                zpool.tile([P, 2, NI], FP8, tag=f"zi{k}", name=f"zi{k}")
                for k in range(kc)
            ]
            zj_sb = [
                zpool.tile([P, 2, N], FP8, tag=f"zj{k}", name=f"zj{k}")
                for k in range(kc)
            ]
            # HWDGE rings exist on sync and scalar only. Order transfers by
            # when the PE consumes them, split across the two queues; the
            # scalar queue is idle until the first ACTIVATE anyway.
            def _zj(eng, k, g):
                c0, c1 = g * CCG, (g + 1) * CCG
                eng.dma_start(zj_sb[k][:, :, c0:c1], zj_t[k, g, :, :, :])

            nc.sync.dma_start(zi_sb[0][:], zi_t[0, :, :, :])
            nc.scalar.dma_start(zi_sb[1][:], zi_t[1, :, :, :])
            _zj(nc.sync, 0, 0)
            _zj(nc.scalar, 1, 0)
            _zj(nc.sync, 0, 1)
            _zj(nc.scalar, 1, 1)
            _zj(nc.sync, 0, 2)
            _zj(nc.scalar, 1, 2)
            _zj(nc.sync, 0, 3)
            _zj(nc.scalar, 1, 3)
        else:
            zi_sb = [
                zpool.tile([P, NI], BF16, tag=f"zi{k}", name=f"zi{k}")
                for k in range(kc)
            ]
            zj_sb = [
                zpool.tile([P, N], BF16, tag=f"zj{k}", name=f"zj{k}")
                for k in range(kc)
            ]
            for k in range(kc):
                nc.sync.dma_start(zi_sb[k][:], zi_t[k * P:(k + 1) * P, :])
            for g in range(NCCG):
                c0, c1 = g * CCG, (g + 1) * CCG
                for k in range(kc):
                    nc.sync.dma_start(
                        zj_sb[k][:, c0:c1], zj_t[k * P:(k + 1) * P, c0:c1]
                    )

        colacc = apool.tile([P, N], BF16, tag="colacc")
        rows_sb = apool.tile([P, RC * NCCG], F32, tag="rows")

        # ---- main loop ------------------------------------------------
        for g in range(NCCG):
            c0 = g * CCG
            for rc in range(RC):
                gp = psump.tile([P, CCG], F32, tag="G")
                for k in range(kc):
                    if fp8:
                        lhsT = zi_sb[k][:, :, rc * P:(rc + 1) * P]
                    else:
                        lhsT = zi_sb[k][:, rc * P:(rc + 1) * P]
                    for cc in range(CCG // MMN):
                        if fp8:
                            rhs = zj_sb[k][:, :, c0 + cc * MMN:c0 + (cc + 1) * MMN]
                        else:
                            rhs = zj_sb[k][:, c0 + cc * MMN:c0 + (cc + 1) * MMN]
                        nc.tensor.matmul(
                            gp[:, cc * MMN:(cc + 1) * MMN],
                            lhsT,
                            rhs,
                            start=(k == 0),
                            stop=(k == kc - 1),
                            perf_mode=perf_mode,
                        )
                et = epool.tile([P, CCG], BF16, tag="E")
                nc.scalar.activation(
                    et[:],
                    gp[:],
                    mybir.ActivationFunctionType.Exp,
                    bias=0.0,
                    scale=exp_scale,
                    accum_out=rows_sb[:, rc * NCCG + g:rc * NCCG + g + 1],
                )
                if rc == 0:
                    nc.vector.tensor_copy(colacc[:, c0:c0 + CCG], et[:])
                else:
                    nc.vector.tensor_add(
                        colacc[:, c0:c0 + CCG], colacc[:, c0:c0 + CCG], et[:]
                    )
            # this group's columns are done — ship them while the next runs
            nc.sync.dma_start(cols_d[:, c0:c0 + CCG], colacc[:, c0:c0 + CCG])

        nc.sync.dma_start(rows_d[:, :], rows_sb[:])


def _get_nc():
    if "nc" not in _compiled:
        _compiled["nc"] = _build()
    return _compiled["nc"]


def _pack_fp8(zt):
    """[D, n] fp32 -> [2, 128, 2, n] fp8 with d = kk*256 + slab*128 + p."""
    n = zt.shape[1]
    return np.ascontiguousarray(
        (zt * FP8_SCALE).reshape(2, 2, P, n).transpose(0, 2, 1, 3)
    ).astype(NP_FP8)


def _pack_fp8_zj(zt):
    """[D, N] fp32 -> [2, NCCG, 128, 2, CCG] fp8: d = kk*256 + slab*128 + p,
    col = g*CCG + c. Each [kk, g] chunk is contiguous for full-rate DMA."""
    return np.ascontiguousarray(
        (zt * FP8_SCALE).reshape(2, 2, P, NCCG, CCG).transpose(0, 3, 2, 1, 4)
    ).astype(NP_FP8)


def _prep_inputs(z_i, z_j):
    """Host-side sharding: normalize (fp32, as the reference), transpose to
    [D, N] (the layout the PE contracts over), quantize, slice per core."""
    zi = np.asarray(z_i, dtype=np.float32)
    zj = np.asarray(z_j, dtype=np.float32)
    ni = np.maximum(np.sqrt((zi * zi).sum(-1, keepdims=True)), EPS)
    nj = np.maximum(np.sqrt((zj * zj).sum(-1, keepdims=True)), EPS)
    zin = zi / ni
    zjn = zj / nj
    pos = (zin * zjn).sum(-1, dtype=np.float64) / TAU  # diagonal of sim, [N]

    zin_t = zin.T  # [D, N]
    zjn_t = zjn.T

    in_maps = []
    if DT_MAIN == "fp8":
        zj_pack = _pack_fp8_zj(zjn_t)
        for c in range(NCORES):
            in_maps.append(
                {
                    "zi_t": _pack_fp8(zin_t[:, c * NI:(c + 1) * NI]),
                    "zj_t": zj_pack,
                }
            )
    else:
        zin_b = np.ascontiguousarray(zin_t.astype(ml_dtypes.bfloat16))
        zjn_b = np.ascontiguousarray(zjn_t.astype(ml_dtypes.bfloat16))
        for c in range(NCORES):
            in_maps.append(
                {
                    "zi_t": np.ascontiguousarray(zin_b[:, c * NI:(c + 1) * NI]),
                    "zj_t": zjn_b,
                }
            )
    return in_maps, pos


def kernel(z_i, z_j):
    global LAST_RESULTS
    in_maps, pos = _prep_inputs(z_i, z_j)
    nc = _get_nc()

    res = bass_utils.run_bass_kernel_spmd(nc, in_maps, core_ids=list(range(NCORES)))
    LAST_RESULTS = res

    rowsum = np.zeros(N, dtype=np.float64)
    colsum = np.zeros(N, dtype=np.float64)
    for c in range(NCORES):
        out = res.results[c]
        rs = out["rowsums"].astype(np.float64)  # [128, RC*NCCG]
        # column rc*NCCG+g holds sum over group g's 2048 cols for row chunk rc
        rs = rs.reshape(P, RC, NCCG).sum(-1)    # [p, rc]
        rowsum[c * NI:(c + 1) * NI] = rs.T.reshape(-1)  # global row = rc*128+p
        colsum += out["colacc"].astype(np.float64).sum(0)

    # host-side "all-reduce" epilogue: drop the diagonal, logs, means
    exp_pos = np.exp(pos)
    lse_row = np.log(rowsum - exp_pos)
    lse_col = np.log(colsum - exp_pos)
    loss_e2t = np.mean(lse_row - pos)
    loss_t2e = np.mean(lse_col - pos)
    loss = 0.5 * (loss_e2t + loss_t2e)
    return np.stack([loss, loss_e2t, loss_t2e]).astype(np.float32)
